# revision 17
# baseline (speedup 1.0000x reference)
"""ListNet-for-Gauss loss kernel for Trainium2 (Bass, raw-scheduled), 8-core SPMD.

Problem: 16384 ranking lists ("segments") of 512 items each (N = 8.4M).
    a = mean + 0.5*variance ; b = mean - 0.5*variance
    per segment s:  S_s = sum(exp(a)), Z_s = sum(exp(t)), W_s = sum(exp(t)*b)
    loss_s = log(S_s) - W_s / Z_s
    output = mean_s(loss_s / seg_len)  (scalar, shape (1,))

Sharding: data-parallel over segments — core c owns segments
[c*2048, (c+1)*2048). The host precomputes a/b (free) and permutes each
core's data into "transposed" tiles: a tile [128, 512] holds element
position r*128+p of segment s at [p, r*512+s]. With the element POSITION
along partitions, the three per-segment sums are partition-dim
reductions, which the Tensor engine does as matmuls against
indicator-ones stationaries — freeing Vector/Scalar from reduction work.

Input planes: a and t ship as fp8e4, b as fp16 (4.2MB/core — the two
cores of an HBM pair stream simultaneously, so per-core input bandwidth
is ~325 GB/s and DMA-engine bytes are the wall; fp8 for b via a casting
SWDGE DMA was tried and is NOT cheaper: the cast transfer costs
engine-time on its fp16 SBUF side). The final loss averages 8.4M terms,
so per-element fp8 noise cancels to ~1e-5 rel err vs the 2e-2 gate.
  e_t: real exp on ACT (fp8 in, fp16 out, [128,2048] instrs).
  e_a: Schraudolph bit-trick exp on DVE — one tensor_scalar computes
       round(a*1477.32 + 15299.7) into int16 whose bit pattern IS fp16
       exp(a) to ~2%; runs in 2x_2p mode even from fp8 input. The
       constant is calibrated so the softmax-weighted bias is ~0.
  w = e_t*b: one 2x fp16 tensor_tensor per block on DVE.

Per-segment sums: group q = 3*sb + plane lands in PSUM partition q via
a [128,9|3] stationary whose column q is ones; 4 accumulating matmuls
per group. Two PSUM banks so sb0-2 stats (bank A) are copied/DMA'd out
while sb3 accumulates into bank B (a PSUM bank is single-ported —
concurrent PE write + ACT read is a hard fault). PE is pre-warmed with
dummy matmuls on a zeroed scratch tile so the HAM clock-gate lifts
(1.2->2.4 GHz) before the real matmuls arrive.

DMA scheduling (measured on this part): per-DMA completion arrives
roughly in ring-byte order at ~stream rate, plus ~1us+ completion tax
per queued DMA — so few, whole-plane DMAs beat tiled ones; issue is
split across the SP HWDGE ring (t+b + stats-out) and the GpSimd SWDGE
ring (a-planes) since one ring issues serially at ~750ns/DMA; t-planes
go early (they gate the exp chain), b3 last (its chain is the shortest:
one TT + 4 matmuls). Every DMA has its own semaphore: cumulative counts
on one semaphore are unsound under per-SDMA-engine completion skew.

The host finishes with log / divide / mean in float64 (negligible).
"""

import sys
import types
from contextlib import ExitStack

import numpy as np
import ml_dtypes

import concourse.mybir as mybir
from concourse import bacc
from concourse.bass_utils import run_bass_kernel_spmd


def _ensure_axon_hooks_shim():
    """bass_utils unconditionally imports antenv.axon_hooks on the trace path;
    some images lack that module. Provide a no-op get/set pair so a stray
    BASS_TRACE=1 degrades to "trace skipped" instead of crashing."""
    try:
        import antenv.axon_hooks  # noqa: F401
        return
    except ImportError:
        pass
    try:
        import antenv
    except ImportError:
        return

    mod = types.ModuleType("antenv.axon_hooks")
    mod._hook = None

    def set_axon_ntff_profile_hook(h):
        mod._hook = h

    def get_axon_ntff_profile_hook():
        return mod._hook

    mod.set_axon_ntff_profile_hook = set_axon_ntff_profile_hook
    mod.get_axon_ntff_profile_hook = get_axon_ntff_profile_hook
    sys.modules["antenv.axon_hooks"] = mod
    antenv.axon_hooks = mod


_ensure_axon_hooks_shim()

N_CORES = 8
NUM_SEG = 16384
SEG_LEN = 512
SEG_PER_CORE = NUM_SEG // N_CORES          # 2048
N_PER_CORE = SEG_PER_CORE * SEG_LEN        # 1048576
P = 128
SB = 4                                     # segment blocks per core
SPB = 512                                  # segments per block
R = SEG_LEN // P                           # 4 partition-rounds per segment
FREE = R * SPB                             # 2048 free elems per tile row
NQ = 3 * SB                                # 12 reduction groups -> PSUM rows

# Schraudolph fp16 exp: bits16(e^a) ~= a*1024*log2(e) + (15 - C)*1024.
# C calibrated (with the fp8 input quantization in the loop) to zero the
# softmax-weighted bias of e_a for a ~ N(0,1) + U(0,1)/2.
SCHR_K = float(1024.0 * np.log2(np.e))
SCHR_C = (15.0 - 0.0589) * 1024.0

F8 = ml_dtypes.float8_e4m3

_CACHE = {}


def _build():
    f8 = mybir.dt.float8e4
    f16 = mybir.dt.float16
    i16 = mybir.dt.int16
    f32 = mybir.dt.float32
    Exp = mybir.ActivationFunctionType.Exp
    mult = mybir.AluOpType.mult
    add = mybir.AluOpType.add

    nc = bacc.Bacc(
        "TRN2",
        target_bir_lowering=False,
        debug=False,
        num_devices=N_CORES,
        detect_race_conditions=False,
    )

    # fp8 rows: per sb, t at (2s)P, a at (2s+1)P. fp16 rows: b at sP.
    xin8_d = nc.dram_tensor("xin8", [2 * SB * P, FREE], f8, kind="ExternalInput")
    xin16_d = nc.dram_tensor("xin16", [SB * P, FREE], f16, kind="ExternalInput")
    st_d = nc.dram_tensor("st_out", [NQ, SPB], f32, kind="ExternalOutput")

    with ExitStack() as ctx:
        sb_t = lambda name, shape, dt: ctx.enter_context(nc.sbuf_tensor(name, shape, dt))
        in_t = [sb_t(f"t{s}", [P, FREE], f8) for s in range(SB)]
        in_a = [sb_t(f"a{s}", [P, FREE], f8) for s in range(SB)]
        in_b = [sb_t(f"b{s}", [P, FREE], f16) for s in range(SB)]
        et_bufs = [sb_t(f"et{s}", [P, FREE], f16) for s in range(SB)]
        ea_bufs = [sb_t(f"ea{s}", [P, FREE], i16) for s in range(SB)]
        w_bufs = [sb_t(f"w{s}", [P, FREE], f16) for s in range(SB)]
        ones_a = sb_t("ones_a", [P, 9 * 9], f16)
        ones_b = sb_t("ones_b", [P, 3 * 3], f16)
        scratch = sb_t("scratch", [P, SPB], f16)
        stats = sb_t("stats", [9, SPB], f32)
        stats2 = sb_t("stats2", [3, SPB], f32)
        psum_a = ctx.enter_context(nc.psum_tensor("acc_a", [9, SPB], f32))
        psum_b = ctx.enter_context(nc.psum_tensor("acc_b", [3, SPB], f32))
        psum_w = ctx.enter_context(nc.psum_tensor("acc_warm", [P, SPB], f32))

        sem = lambda name: ctx.enter_context(nc.semaphore(name))
        d_t = [sem(f"d_t{s}") for s in range(SB)]
        d_a = [sem(f"d_a{s}") for s in range(SB)]
        d_b = [sem(f"d_b{s}") for s in range(SB)]
        s_scr = sem("s_scr")
        s_ones = sem("s_ones")
        s_et = sem("s_et")
        s_ea = sem("s_ea")
        s_w = sem("s_w")
        s_pe = sem("s_pe")
        s_copy = sem("s_copy")
        out_sem = sem("out_sem")

        with nc.Block() as block:

            @block.sync
            def _(sync):
                # t-planes early (they gate exp_t -> TT -> W chains); b3
                # last: its dependent chain (one TT + 4 matmuls) is the
                # shortest. Whole-plane DMAs only — completion arrives in
                # ring order plus a per-DMA tax, so fewer DMAs win.
                order = [
                    ("t", 0), ("t", 1), ("b", 0), ("t", 2), ("b", 1),
                    ("t", 3), ("b", 2), ("b", 3),
                ]
                for kind, s in order:
                    if kind == "t":
                        sync.dma_start(
                            out=in_t[s][:], in_=xin8_d[2 * s * P : (2 * s + 1) * P, :]
                        ).then_inc(d_t[s], 16)
                    else:
                        sync.dma_start(
                            out=in_b[s][:], in_=xin16_d[s * P : (s + 1) * P, :]
                        ).then_inc(d_b[s], 16)
                sync.wait_ge(s_copy, 1)
                sync.dma_start(out=st_d[0:9, :], in_=stats[:, :]).then_inc(out_sem, 16)
                sync.wait_ge(s_copy, 2)
                sync.dma_start(out=st_d[9:NQ, :], in_=stats2[:, :]).then_inc(out_sem, 16)
                sync.wait_ge(out_sem, 32)

            @block.gpsimd
            def _(gpsimd):
                for s in range(SB):
                    gpsimd.dma_start(
                        out=in_a[s][:], in_=xin8_d[(2 * s + 1) * P : (2 * s + 2) * P, :]
                    ).then_inc(d_a[s], 16)

            @block.scalar
            def _(scalar):
                for s in range(SB):
                    scalar.wait_ge(d_t[s], 16)
                    nc.scalar.activation(et_bufs[s][:], in_t[s][:], Exp).then_inc(s_et, 1)
                scalar.wait_ge(s_pe, 9)
                nc.scalar.copy(stats[:, :], psum_a[:, :]).then_inc(s_copy, 1)
                scalar.wait_ge(s_pe, NQ)
                nc.scalar.copy(stats2[:, :], psum_b[:, :]).then_inc(s_copy, 1)

            @block.vector
            def _(vector):
                # scratch first: it gates the PE warmup matmuls.
                nc.vector.memset(scratch[:], 0.0).then_inc(s_scr, 1)
                # Indicator stationaries: block q of ones_a ([128,9] at col
                # 9q) is zero except column q (abs col 10q) = 1; ones_b
                # likewise ([128,3] blocks, one at abs col 4j).
                nc.vector.memset(ones_a[:], 0.0)
                nc.vector.memset(ones_b[:], 0.0)
                for q in range(9):
                    nc.vector.memset(ones_a[:, 10 * q : 10 * q + 1], 1.0)
                last = None
                for j in range(3):
                    last = nc.vector.memset(ones_b[:, 4 * j : 4 * j + 1], 1.0)
                last.then_inc(s_ones, 1)
                # a-planes arrive early on the pool ring: all four TS first,
                # then the TTs in b-arrival order.
                for s in range(SB):
                    vector.wait_ge(d_a[s], 16)
                    nc.vector.tensor_scalar(
                        ea_bufs[s][:], in_a[s][:], SCHR_K, SCHR_C, mult, add
                    ).then_inc(s_ea, 1)
                for s in range(SB):
                    vector.wait_ge(s_et, s + 1)
                    vector.wait_ge(d_b[s], 16)
                    nc.vector.tensor_tensor(
                        w_bufs[s][:], et_bufs[s][:], in_b[s][:], mult
                    ).then_inc(s_w, 1)

            @block.tensor
            def _(tensor):
                # HAM warmup: cold matmuls of zeros keep PE busy through an
                # activity window so it runs at 2.4 GHz for the real work.
                tensor.wait_ge(s_scr, 1)
                for _ in range(6):
                    nc.tensor.matmul(
                        out=psum_w[:, :],
                        lhsT=scratch[:, 0:P],
                        rhs=scratch[:, :],
                        start=True,
                        stop=True,
                        skip_group_check=True,
                    )
                tensor.wait_ge(s_ones, 1)

                first = {"a": True, "b": True}

                def group(q, bank, rhs_slc, wait_sem, wait_val, stop):
                    tensor.wait_ge(wait_sem, wait_val)
                    out_ap = psum_a[:, :] if bank == "a" else psum_b[:, :]
                    if bank == "a":
                        lhsT = ones_a[:, 9 * q : 9 * (q + 1)]
                    else:
                        j = q - 9
                        lhsT = ones_b[:, 3 * j : 3 * (j + 1)]
                    mm = None
                    for r in range(R):
                        mm = nc.tensor.matmul(
                            out=out_ap,
                            lhsT=lhsT,
                            rhs=rhs_slc(r),
                            start=first[bank],
                            stop=(stop and r == R - 1),
                            skip_group_check=True,
                        )
                        first[bank] = False
                    mm.then_inc(s_pe, 1)

                def ea_slc(s):
                    return lambda r: ea_bufs[s][:, r * SPB : (r + 1) * SPB].bitcast(
                        mybir.dt.float16
                    )

                def et_slc(s):
                    return lambda r: et_bufs[s][:, r * SPB : (r + 1) * SPB]

                def w_slc(s):
                    return lambda r: w_bufs[s][:, r * SPB : (r + 1) * SPB]

                # per sb: S (a, earliest), Z (t), W (w, latest). Rows stay
                # Z=3s, W=3s+1, S=3s+2 via the stationary block choice.
                for s in range(SB - 1):
                    group(3 * s + 2, "a", ea_slc(s), s_ea, s + 1, stop=False)
                    group(3 * s + 0, "a", et_slc(s), s_et, s + 1, stop=False)
                    group(3 * s + 1, "a", w_slc(s), s_w, s + 1, stop=(s == SB - 2))
                s3 = SB - 1
                group(11, "b", ea_slc(s3), s_ea, SB, stop=False)
                group(9, "b", et_slc(s3), s_et, SB, stop=False)
                group(10, "b", w_slc(s3), s_w, SB, stop=True)

        nc.compile()
    return nc


# test.py reads this for the neuron-profile exec time (BASS_TRACE=1).
last_results = None


def _pack_plane(arr):
    """[2048 segs, 512 elems] -> [SB, 128, FREE] transposed tiles."""
    out = np.empty((SB, P, FREE), dtype=arr.dtype)
    for s in range(SB):
        blk = arr[s * SPB : (s + 1) * SPB]              # [512s, 512e]
        out[s] = blk.reshape(SPB, R, P).transpose(2, 1, 0).reshape(P, FREE)
    return out


def kernel(mean, variance, scope, targets):
    global last_results
    if "nc" not in _CACHE:
        _CACHE["nc"] = _build()
    nc = _CACHE["nc"]

    x = np.asarray(mean, dtype=np.float32).reshape(-1)
    y = np.asarray(variance, dtype=np.float32).reshape(-1)
    t = np.asarray(targets, dtype=np.float32).reshape(-1)
    a8 = (x + 0.5 * y).astype(F8)
    t8 = t.astype(F8)
    b16 = (x - 0.5 * y).astype(np.float16)

    in_maps = []
    for c in range(N_CORES):
        lo, hi = c * N_PER_CORE, (c + 1) * N_PER_CORE
        pt = _pack_plane(t8[lo:hi].reshape(SEG_PER_CORE, SEG_LEN))
        pa = _pack_plane(a8[lo:hi].reshape(SEG_PER_CORE, SEG_LEN))
        pb = _pack_plane(b16[lo:hi].reshape(SEG_PER_CORE, SEG_LEN))
        xin8 = np.empty((2 * SB, P, FREE), dtype=F8)
        xin8[0::2] = pt
        xin8[1::2] = pa
        in_maps.append(
            {
                "xin8": np.ascontiguousarray(xin8.reshape(2 * SB * P, FREE)),
                "xin16": np.ascontiguousarray(pb.reshape(SB * P, FREE)),
            }
        )

    res = run_bass_kernel_spmd(nc, in_maps, core_ids=list(range(N_CORES)))
    last_results = res

    seg_len = np.asarray(scope, dtype=np.float64).reshape(-1)
    total = 0.0
    for c in range(N_CORES):
        out = res.results[c]["st_out"].astype(np.float64)  # [12, 512]
        Z = out[0::3].reshape(-1)
        W = out[1::3].reshape(-1)
        S = out[2::3].reshape(-1)
        sc = seg_len[c * SEG_PER_CORE : (c + 1) * SEG_PER_CORE]
        total += float(np.sum((np.log(S) - W / Z) / sc))
    return np.asarray([total / NUM_SEG], dtype=np.float32)


# revision 20
# speedup vs baseline: 1.2308x; 1.2308x over previous
"""ListNet-for-Gauss loss kernel for Trainium2 (Bass, raw-scheduled), 8-core SPMD.

Problem: 16384 ranking lists ("segments") of 512 items each (N = 8.4M).
    a = mean + 0.5*variance ; b = mean - 0.5*variance
    per segment s:  S_s = sum(exp(a)), Z_s = sum(exp(t)), W_s = sum(exp(t)*b)
    loss_s = log(S_s) - W_s / Z_s
    output = mean_s(loss_s / seg_len)  (scalar, shape (1,))

Sharding: data-parallel over segments — core c owns segments
[c*2048, (c+1)*2048). The host precomputes a/b (free) and permutes each
core's data into "transposed" tiles: a tile [128, 512] holds element
position r*128+p of segment s at [p, r*512+s]. With the element POSITION
along partitions, the three per-segment sums are partition-dim
reductions, which the Tensor engine does as matmuls against
indicator-ones stationaries — freeing Vector/Scalar from reduction work.

All three planes ship as fp8e4 (3.1MB/core; the two cores of an HBM pair
stream simultaneously, so per-core input bandwidth is ~325 GB/s and DMA
bytes are the wall — the final loss averages 8.4M terms, so fp8 noise
cancels to ~3e-5 rel err vs the 2e-2 gate). b is consumed as fp8
directly by the w-multiply (1x DVE mode — cheaper overall than shipping
fp16 b or DMA-casting, both of which cost DMA-engine bytes).
  e_t: real exp on ACT (fp8 in, fp16 out, [128,2048] instrs).
  e_a: Schraudolph bit-trick exp on DVE — one tensor_scalar computes
       round(a*1477.32 + 15299.7) into int16 whose bit pattern IS fp16
       exp(a) to ~2%; runs in 2x_2p mode even from fp8 input. The
       constant is calibrated so the softmax-weighted bias is ~0.

Per-segment sums: group q = 3*sb + plane lands in PSUM partition q via
a [128,9|3] stationary whose column q is ones; 4 accumulating matmuls
per group. Two PSUM banks so sb0-2 stats (bank A) are copied/DMA'd out
while sb3 accumulates into bank B (a PSUM bank is single-ported —
concurrent PE write + ACT read is a hard fault). PE is pre-warmed with
dummy matmuls on a zeroed scratch tile so the HAM clock-gate lifts
(1.2->2.4 GHz) before the real matmuls arrive.

DMA scheduling (measured on this part): per-DMA completions arrive in
ring-byte order at the shared drain rate plus a per-DMA completion tax,
so the SP ring interleaves t,b per block (matching PE's in-order
consumption: Z, W, S per block) while the GpSimd SWDGE ring carries the
a-planes; every DMA has its own semaphore (cumulative counts on one
semaphore are unsound under per-SDMA-engine completion skew).

The host finishes with log / divide / mean in float64 (negligible).
"""

import sys
import types
from contextlib import ExitStack

import numpy as np
import ml_dtypes

import concourse.mybir as mybir
from concourse import bacc
from concourse.bass_utils import run_bass_kernel_spmd


def _ensure_axon_hooks_shim():
    """bass_utils unconditionally imports antenv.axon_hooks on the trace path;
    some images lack that module. Provide a no-op get/set pair so a stray
    BASS_TRACE=1 degrades to "trace skipped" instead of crashing."""
    try:
        import antenv.axon_hooks  # noqa: F401
        return
    except ImportError:
        pass
    try:
        import antenv
    except ImportError:
        return

    mod = types.ModuleType("antenv.axon_hooks")
    mod._hook = None

    def set_axon_ntff_profile_hook(h):
        mod._hook = h

    def get_axon_ntff_profile_hook():
        return mod._hook

    mod.set_axon_ntff_profile_hook = set_axon_ntff_profile_hook
    mod.get_axon_ntff_profile_hook = get_axon_ntff_profile_hook
    sys.modules["antenv.axon_hooks"] = mod
    antenv.axon_hooks = mod


_ensure_axon_hooks_shim()

N_CORES = 8
NUM_SEG = 16384
SEG_LEN = 512
SEG_PER_CORE = NUM_SEG // N_CORES          # 2048
N_PER_CORE = SEG_PER_CORE * SEG_LEN        # 1048576
P = 128
SB = 4                                     # segment blocks per core
SPB = 512                                  # segments per block
R = SEG_LEN // P                           # 4 partition-rounds per segment
FREE = R * SPB                             # 2048 free elems per tile row
NQ = 3 * SB                                # 12 reduction groups -> PSUM rows

# Schraudolph fp16 exp: bits16(e^a) ~= a*1024*log2(e) + (15 - C)*1024.
# C calibrated (with the fp8 input quantization in the loop) to zero the
# softmax-weighted bias of e_a for a ~ N(0,1) + U(0,1)/2.
SCHR_K = float(1024.0 * np.log2(np.e))
SCHR_C = (15.0 - 0.0589) * 1024.0

F8 = ml_dtypes.float8_e4m3

_CACHE = {}


def _build():
    f8 = mybir.dt.float8e4
    f16 = mybir.dt.float16
    i16 = mybir.dt.int16
    f32 = mybir.dt.float32
    Exp = mybir.ActivationFunctionType.Exp
    mult = mybir.AluOpType.mult
    add = mybir.AluOpType.add

    nc = bacc.Bacc(
        "TRN2",
        target_bir_lowering=False,
        debug=False,
        num_devices=N_CORES,
        detect_race_conditions=False,
    )

    # rows per sb: t at (3s)P, b at (3s+1)P, a at (3s+2)P
    xin_d = nc.dram_tensor("xin8", [3 * SB * P, FREE], f8, kind="ExternalInput")
    st_d = nc.dram_tensor("st_out", [NQ, SPB], f32, kind="ExternalOutput")

    def t_rows(s):
        return xin_d[3 * s * P : (3 * s + 1) * P, :]

    def b_rows(s):
        return xin_d[(3 * s + 1) * P : (3 * s + 2) * P, :]

    def a_rows(s):
        return xin_d[(3 * s + 2) * P : (3 * s + 3) * P, :]

    with ExitStack() as ctx:
        sb_t = lambda name, shape, dt: ctx.enter_context(nc.sbuf_tensor(name, shape, dt))
        in_t = [sb_t(f"t{s}", [P, FREE], f8) for s in range(SB)]
        in_a = [sb_t(f"a{s}", [P, FREE], f8) for s in range(SB)]
        in_b = [sb_t(f"b{s}", [P, FREE], f8) for s in range(SB)]
        et_bufs = [sb_t(f"et{s}", [P, FREE], f16) for s in range(SB)]
        ea_bufs = [sb_t(f"ea{s}", [P, FREE], i16) for s in range(SB)]
        w_bufs = [sb_t(f"w{s}", [P, FREE], f16) for s in range(SB)]
        ones_a = sb_t("ones_a", [P, 9 * 9], f16)
        ones_b = sb_t("ones_b", [P, 3 * 3], f16)
        scratch = sb_t("scratch", [P, SPB], f16)
        stats = sb_t("stats", [9, SPB], f32)
        stats2 = sb_t("stats2", [3, SPB], f32)
        psum_a = ctx.enter_context(nc.psum_tensor("acc_a", [9, SPB], f32))
        psum_b = ctx.enter_context(nc.psum_tensor("acc_b", [3, SPB], f32))
        psum_w = ctx.enter_context(nc.psum_tensor("acc_warm", [P, SPB], f32))

        sem = lambda name: ctx.enter_context(nc.semaphore(name))
        d_t = [sem(f"d_t{s}") for s in range(SB)]
        d_b = [sem(f"d_b{s}") for s in range(SB)]
        d_a = [sem(f"d_a{s}") for s in range(SB)]
        s_scr = sem("s_scr")
        s_ones = sem("s_ones")
        s_et = sem("s_et")
        s_ea = sem("s_ea")
        s_w = sem("s_w")
        s_pe = sem("s_pe")
        s_copy = sem("s_copy")
        out_sem = sem("out_sem")

        with nc.Block() as block:

            @block.sync
            def _(sync):
                # Interleave t,b per block so completions (ring-byte order)
                # match PE's per-block consumption order.
                for s in range(SB):
                    sync.dma_start(out=in_t[s][:], in_=t_rows(s)).then_inc(d_t[s], 16)
                    sync.dma_start(out=in_b[s][:], in_=b_rows(s)).then_inc(d_b[s], 16)
                sync.wait_ge(s_copy, 1)
                sync.dma_start(out=st_d[0:9, :], in_=stats[:, :]).then_inc(out_sem, 16)
                sync.wait_ge(s_copy, 2)
                sync.dma_start(out=st_d[9:NQ, :], in_=stats2[:, :]).then_inc(out_sem, 16)
                sync.wait_ge(out_sem, 32)

            @block.gpsimd
            def _(gpsimd):
                for s in range(SB):
                    gpsimd.dma_start(out=in_a[s][:], in_=a_rows(s)).then_inc(d_a[s], 16)

            @block.scalar
            def _(scalar):
                for s in range(SB):
                    scalar.wait_ge(d_t[s], 16)
                    nc.scalar.activation(et_bufs[s][:], in_t[s][:], Exp).then_inc(s_et, 1)
                scalar.wait_ge(s_pe, 9)
                nc.scalar.copy(stats[:, :], psum_a[:, :]).then_inc(s_copy, 1)
                scalar.wait_ge(s_pe, NQ)
                nc.scalar.copy(stats2[:, :], psum_b[:, :]).then_inc(s_copy, 1)

            @block.vector
            def _(vector):
                # scratch first: it gates the PE warmup matmuls.
                nc.vector.memset(scratch[:], 0.0).then_inc(s_scr, 1)
                # Indicator stationaries: block q of ones_a ([128,9] at col
                # 9q) is zero except column q (abs col 10q) = 1; ones_b
                # likewise ([128,3] blocks, one at abs col 4j).
                nc.vector.memset(ones_a[:], 0.0)
                nc.vector.memset(ones_b[:], 0.0)
                for q in range(9):
                    nc.vector.memset(ones_a[:, 10 * q : 10 * q + 1], 1.0)
                last = None
                for j in range(3):
                    last = nc.vector.memset(ones_b[:, 4 * j : 4 * j + 1], 1.0)
                last.then_inc(s_ones, 1)
                for s in range(SB):
                    vector.wait_ge(d_a[s], 16)
                    nc.vector.tensor_scalar(
                        ea_bufs[s][:], in_a[s][:], SCHR_K, SCHR_C, mult, add
                    ).then_inc(s_ea, 1)
                    vector.wait_ge(s_et, s + 1)
                    vector.wait_ge(d_b[s], 16)
                    nc.vector.tensor_tensor(
                        w_bufs[s][:], et_bufs[s][:], in_b[s][:], mult
                    ).then_inc(s_w, 1)

            @block.tensor
            def _(tensor):
                # HAM warmup: cold matmuls of zeros keep PE busy through an
                # activity window so it runs at 2.4 GHz for the real work.
                tensor.wait_ge(s_scr, 1)
                for _ in range(6):
                    nc.tensor.matmul(
                        out=psum_w[:, :],
                        lhsT=scratch[:, 0:P],
                        rhs=scratch[:, :],
                        start=True,
                        stop=True,
                        skip_group_check=True,
                    )
                tensor.wait_ge(s_ones, 1)

                first = {"a": True, "b": True}

                def group(q, bank, rhs_slc, wait_sem, wait_val, stop):
                    tensor.wait_ge(wait_sem, wait_val)
                    out_ap = psum_a[:, :] if bank == "a" else psum_b[:, :]
                    if bank == "a":
                        lhsT = ones_a[:, 9 * q : 9 * (q + 1)]
                    else:
                        j = q - 9
                        lhsT = ones_b[:, 3 * j : 3 * (j + 1)]
                    mm = None
                    for r in range(R):
                        mm = nc.tensor.matmul(
                            out=out_ap,
                            lhsT=lhsT,
                            rhs=rhs_slc(r),
                            start=first[bank],
                            stop=(stop and r == R - 1),
                            skip_group_check=True,
                        )
                        first[bank] = False
                    mm.then_inc(s_pe, 1)

                def ea_slc(s):
                    return lambda r: ea_bufs[s][:, r * SPB : (r + 1) * SPB].bitcast(
                        mybir.dt.float16
                    )

                def et_slc(s):
                    return lambda r: et_bufs[s][:, r * SPB : (r + 1) * SPB]

                def w_slc(s):
                    return lambda r: w_bufs[s][:, r * SPB : (r + 1) * SPB]

                # per sb: Z (t), S (a), W (w, last — its post-arrival chain
                # is the shortest). Rows stay Z=3s, W=3s+1, S=3s+2 via the
                # stationary block choice.
                for s in range(SB - 1):
                    group(3 * s + 0, "a", et_slc(s), s_et, s + 1, stop=False)
                    group(3 * s + 2, "a", ea_slc(s), s_ea, s + 1, stop=False)
                    group(3 * s + 1, "a", w_slc(s), s_w, s + 1, stop=(s == SB - 2))
                s3 = SB - 1
                group(9, "b", et_slc(s3), s_et, SB, stop=False)
                group(11, "b", ea_slc(s3), s_ea, SB, stop=False)
                group(10, "b", w_slc(s3), s_w, SB, stop=True)

        nc.compile()
    return nc


# test.py reads this for the neuron-profile exec time (BASS_TRACE=1).
last_results = None


def _pack_plane(arr):
    """[2048 segs, 512 elems] -> [SB, 128, FREE] transposed tiles."""
    out = np.empty((SB, P, FREE), dtype=arr.dtype)
    for s in range(SB):
        blk = arr[s * SPB : (s + 1) * SPB]              # [512s, 512e]
        out[s] = blk.reshape(SPB, R, P).transpose(2, 1, 0).reshape(P, FREE)
    return out


def kernel(mean, variance, scope, targets):
    global last_results
    if "nc" not in _CACHE:
        _CACHE["nc"] = _build()
    nc = _CACHE["nc"]

    x = np.asarray(mean, dtype=np.float32).reshape(-1)
    y = np.asarray(variance, dtype=np.float32).reshape(-1)
    t = np.asarray(targets, dtype=np.float32).reshape(-1)
    a8 = (x + 0.5 * y).astype(F8)
    t8 = t.astype(F8)
    b8 = (x - 0.5 * y).astype(F8)

    in_maps = []
    for c in range(N_CORES):
        lo, hi = c * N_PER_CORE, (c + 1) * N_PER_CORE
        pt = _pack_plane(t8[lo:hi].reshape(SEG_PER_CORE, SEG_LEN))
        pb = _pack_plane(b8[lo:hi].reshape(SEG_PER_CORE, SEG_LEN))
        pa = _pack_plane(a8[lo:hi].reshape(SEG_PER_CORE, SEG_LEN))
        xin = np.empty((3 * SB, P, FREE), dtype=F8)
        xin[0::3] = pt
        xin[1::3] = pb
        xin[2::3] = pa
        in_maps.append({"xin8": np.ascontiguousarray(xin.reshape(3 * SB * P, FREE))})

    res = run_bass_kernel_spmd(nc, in_maps, core_ids=list(range(N_CORES)))
    last_results = res

    seg_len = np.asarray(scope, dtype=np.float64).reshape(-1)
    total = 0.0
    for c in range(N_CORES):
        out = res.results[c]["st_out"].astype(np.float64)  # [12, 512]
        Z = out[0::3].reshape(-1)
        W = out[1::3].reshape(-1)
        S = out[2::3].reshape(-1)
        sc = seg_len[c * SEG_PER_CORE : (c + 1) * SEG_PER_CORE]
        total += float(np.sum((np.log(S) - W / Z) / sc))
    return np.asarray([total / NUM_SEG], dtype=np.float32)
